# revision 7
# baseline (speedup 1.0000x reference)
"""Cross-attention (GroupNorm + 1x1-conv QKV + softmax attention + out-proj
+ residual) for B=2, C=256, H=W=64, 4 heads, on 8 Trainium2 NeuronCores.

Sharding: core i handles (batch b = i//4, head h = i%4) — data-parallel over
batch x tensor-parallel over heads. Each core computes its head's attention
output projected through its slice of wout columns (a partial sum over the
full [256, 64*64] output); the host sums the 4 head-partials per batch and
adds wout bias + residual input.

Device math per core:
  GN stats via bn_stats per channel + PE group-aggregation matmuls (fp32r);
  the GN affine is folded into the QKV weights (w' = wT * scale_ch, bias
  columns via wT @ shift matmuls) so normalized tensors are never
  materialized. All high-volume matmuls (projections, S^T = k^T q,
  attention*V, out-proj) run bf16 with fp32 PSUM accumulation. S^T is
  computed directly in [k, q] layout; softmax needs no max-subtraction
  (scores are O(1): GN-normalized inputs, 1/sqrt(C) scaling — verified
  |s| < 4) and the denominator comes free from a ones-column appended to
  v^T in the attention-value matmul. The softmax normalization is applied
  after the out-projection (scales commute past the column-wise matmul).
"""

import numpy as np

import concourse.bass as bass
import concourse.tile as tile
from concourse import bacc, mybir
from concourse.bass_utils import run_bass_kernel_spmd

F32 = mybir.dt.float32
F32R = mybir.dt.float32r
BF16 = mybir.dt.bfloat16
AF = mybir.ActivationFunctionType

B, C, HH, WW = 2, 256, 64, 64
NH, HD, NG = 4, 64, 32          # heads, head_dim, groups
S = HH * WW                      # 4096 spatial positions
CT = C // 128                    # channel tiles (2)
EPS = 1e-5
SCALE = 1.0 / np.sqrt(np.float32(C))  # attention scale 1/16
KT = S // 128                    # 32 k-tiles
QSB = 1024                       # q superblock (exp/psum tile width)
NQSB = S // QSB                  # 4
JB = QSB // 512                  # 2 512-blocks per superblock

_compiled = None


def _build():
    nc = bacc.Bacc()

    x_d = nc.dram_tensor("x", [C, S], F32, kind="ExternalInput")
    c_d = nc.dram_tensor("cx", [C, S], F32, kind="ExternalInput")
    gamma_d = nc.dram_tensor("gamma", [C, 1], F32, kind="ExternalInput")
    beta_d = nc.dram_tensor("beta", [C, 1], F32, kind="ExternalInput")
    wqT_d = nc.dram_tensor("wqT", [C, HD], F32, kind="ExternalInput")
    wkT_d = nc.dram_tensor("wkT", [C, HD], F32, kind="ExternalInput")
    wvT_d = nc.dram_tensor("wvT", [C, HD], F32, kind="ExternalInput")
    woT_d = nc.dram_tensor("woT", [HD, C], F32, kind="ExternalInput")
    gagg_d = nc.dram_tensor("gagg", [CT, 128, NG], F32, kind="ExternalInput")
    gbc_d = nc.dram_tensor("gbc", [CT, NG, 128], F32, kind="ExternalInput")
    ones_d = nc.dram_tensor("onesd", [1, 512], F32, kind="ExternalInput")
    out_d = nc.dram_tensor("out", [C, S], F32, kind="ExternalOutput")
    rscr_d = nc.dram_tensor("rscr", [NQSB, QSB], F32)  # recip bounce buffer

    with tile.TileContext(nc) as tc:
        with tc.tile_pool(name="cst", bufs=1) as cst, \
             tc.tile_pool(name="work", bufs=2) as work, \
             tc.tile_pool(name="ppool", bufs=5) as ppool, \
             tc.tile_pool(name="ostage", bufs=4) as ostage, \
             tc.tile_pool(name="sp", bufs=2, space="PSUM") as sp, \
             tc.tile_pool(name="op", bufs=1, space="PSUM") as op, \
             tc.tile_pool(name="pp", bufs=2, space="PSUM") as pp:

            # ---------- loads (x/c first: GN stats are the critical path) ----------
            # f32 via fast HWDGE queues (chunked for earlier stats start);
            # bf16 copies for the PE made on ACT/DVE (gpsimd cast-DMA is slow)
            x_f = cst.tile([128, CT, S], F32)
            c_f = cst.tile([128, CT, S], F32)
            for ct in range(CT):
                for ch in range(2):
                    nc.sync.dma_start(out=x_f[:, ct, bass.ts(ch, S // 2)],
                                      in_=x_d[ct * 128:(ct + 1) * 128, bass.ts(ch, S // 2)])
            for ct in range(CT):
                for ch in range(2):
                    nc.sync.dma_start(out=c_f[:, ct, bass.ts(ch, S // 2)],
                                      in_=c_d[ct * 128:(ct + 1) * 128, bass.ts(ch, S // 2)])
            x_sb = cst.tile([128, CT, S], BF16)
            c_sb = cst.tile([128, CT, S], BF16)
            for ct in range(CT):
                nc.scalar.copy(out=x_sb[:, ct, :], in_=x_f[:, ct, :])
                nc.vector.tensor_copy(out=c_sb[:, ct, :], in_=c_f[:, ct, :])
            wqT = cst.tile([128, CT, HD], F32R)
            wkT = cst.tile([128, CT, HD], F32R)
            wvT = cst.tile([128, CT, HD], F32R)
            for ct in range(CT):
                nc.gpsimd.dma_start(out=wqT[:, ct, :], in_=wqT_d[ct * 128:(ct + 1) * 128, :])
                nc.gpsimd.dma_start(out=wkT[:, ct, :], in_=wkT_d[ct * 128:(ct + 1) * 128, :])
                nc.gpsimd.dma_start(out=wvT[:, ct, :], in_=wvT_d[ct * 128:(ct + 1) * 128, :])
            woT = cst.tile([HD, CT, 128], BF16)
            for ct in range(CT):
                nc.gpsimd.dma_start(out=woT[:, ct, :], in_=woT_d[:, ct * 128:(ct + 1) * 128])
            gagg = cst.tile([128, CT, NG], F32R)
            gbc = cst.tile([NG, CT, 128], F32R)
            for ct in range(CT):
                nc.gpsimd.dma_start(out=gagg[:, ct, :], in_=gagg_d[ct, :, :])
                nc.gpsimd.dma_start(out=gbc[:, ct, :], in_=gbc_d[ct, :, :])
            ones_sb = cst.tile([1, 512], BF16)
            nc.gpsimd.dma_start(out=ones_sb, in_=ones_d[:, :])
            gamma_sb = cst.tile([128, CT, 1], F32)
            beta_sb = cst.tile([128, CT, 1], F32)
            for ct in range(CT):
                nc.sync.dma_start(out=gamma_sb[:, ct, :], in_=gamma_d[ct * 128:(ct + 1) * 128, :])
                nc.sync.dma_start(out=beta_sb[:, ct, :], in_=beta_d[ct * 128:(ct + 1) * 128, :])
            eps_t = cst.tile([NG, 1], F32)
            nc.vector.memset(eps_t, EPS)

            # ---------- GroupNorm stats -> per-channel scale/shift ----------
            def gn_affine(src_sb, name):
                # per-channel mean/var over spatial (8 bn_stats subgroups of 512)
                st = cst.tile([128, CT, 8, 6], F32, tag=f"st_{name}")
                mv = cst.tile([128, CT, 2], F32, tag=f"mv_{name}")
                mt = cst.tile([128, CT, 2], F32R, tag=f"mt_{name}")
                for ct in range(CT):
                    for sg in range(8):
                        nc.vector.bn_stats(out=st[:, ct, sg, :],
                                           in_=src_sb[:, ct, sg * 512:(sg + 1) * 512])
                    nc.vector.bn_aggr(out=mv[:, ct, :], in_=st[:, ct, :, :])
                    # mt = [m, v + m^2]  (rounded to f32r for the PE aggregation)
                    nc.vector.tensor_mul(out=mt[:, ct, 1:2], in0=mv[:, ct, 0:1], in1=mv[:, ct, 0:1])
                    nc.vector.tensor_add(out=mt[:, ct, 1:2],
                                         in0=mt[:, ct, 1:2].bitcast(F32), in1=mv[:, ct, 1:2])
                    nc.vector.tensor_copy(out=mt[:, ct, 0:1], in_=mv[:, ct, 0:1])
                # group sums over the 8 channels of each group
                gs = pp.tile([NG, 2], F32, tag="pp")
                for ct in range(CT):
                    nc.tensor.matmul(gs[:, :], gagg[:, ct, :], mt[:, ct, :],
                                     start=(ct == 0), stop=(ct == CT - 1))
                grp = cst.tile([NG, 2], F32, tag=f"grp_{name}")
                nc.scalar.mul(out=grp, in_=gs[:, :], mul=1.0 / (C // NG))  # [Mg, E[x^2]g]
                vg = cst.tile([NG, 1], F32, tag=f"vg_{name}")
                nc.vector.tensor_mul(out=vg, in0=grp[:, 0:1], in1=grp[:, 0:1])
                nc.vector.tensor_sub(out=vg, in0=grp[:, 1:2], in1=vg)
                nc.scalar.activation(out=vg, in_=vg, func=AF.Sqrt, bias=eps_t)
                grp2 = cst.tile([NG, 2], F32R, tag=f"grp2_{name}")
                with nc.allow_low_precision(reason="f32r keeps ~fp32 precision"):
                    nc.vector.reciprocal(out=grp2[:, 0:1], in_=vg)  # rstd_g
                nc.vector.tensor_copy(out=grp2[:, 1:2], in_=grp[:, 0:1])  # Mg
                # broadcast group -> channel, then channel affine
                scale = cst.tile([128, CT, 1], F32, tag=f"scale_{name}")
                shift = cst.tile([128, CT, 1], F32R, tag=f"shift_{name}")
                for ct in range(CT):
                    cb = pp.tile([128, 2], F32, tag="pp")
                    nc.tensor.matmul(cb[:, :], gbc[:, ct, :], grp2[:, :],
                                     start=True, stop=True)
                    nc.vector.tensor_mul(out=scale[:, ct, :], in0=gamma_sb[:, ct, :], in1=cb[:, 0:1])
                    tmp = work.tile([128, 1], F32, tag="gn_tmp")
                    nc.vector.tensor_mul(out=tmp, in0=cb[:, 1:2], in1=scale[:, ct, :])
                    nc.vector.tensor_sub(out=shift[:, ct, :], in0=beta_sb[:, ct, :], in1=tmp)
                return scale, shift

            scale_x, shift_x = gn_affine(x_f, "x")
            scale_c, shift_c = gn_affine(c_f, "c")

            # ---------- fold GN affine into weights (bf16 for the PE) ----------
            wq2 = cst.tile([128, CT, HD], BF16)
            wk2 = cst.tile([128, CT, HD], BF16)
            wv2 = cst.tile([128, CT, HD], BF16)
            for ct in range(CT):
                nc.vector.tensor_scalar_mul(out=wq2[:, ct, :], in0=wqT[:, ct, :].bitcast(F32),
                                            scalar1=scale_x[:, ct, :])
                nc.vector.tensor_scalar_mul(out=wk2[:, ct, :], in0=wkT[:, ct, :].bitcast(F32),
                                            scalar1=scale_c[:, ct, :])
                nc.vector.tensor_scalar_mul(out=wv2[:, ct, :], in0=wvT[:, ct, :].bitcast(F32),
                                            scalar1=scale_c[:, ct, :])

            # q/k bias as per-partition columns (added during PSUM evacuation):
            # row matmul -> [1, HD], then a tiny DMA scatter to [HD, 1]
            def bias_col(wT, shift, name):
                bp = pp.tile([1, HD], F32, tag="pp")
                for ct in range(CT):
                    nc.tensor.matmul(bp[:, :], shift[:, ct, :], wT[:, ct, :],
                                     start=(ct == 0), stop=(ct == CT - 1))
                brow = cst.tile([1, HD], F32, tag=f"brow_{name}")
                nc.vector.tensor_copy(out=brow, in_=bp[:, :])
                bcol = cst.tile([HD, 1], F32, tag=f"bcol_{name}")
                nc.sync.dma_start(out=bcol, in_=brow.rearrange("p (a b) -> p a b", a=HD))
                return bcol

            qbc = bias_col(wqT, shift_x, "q")
            kbc = bias_col(wkT, shift_c, "k")
            # v bias as a row (added via a K=1 matmul in the v^T projection)
            vbp = pp.tile([1, HD], F32, tag="pp")
            for ct in range(CT):
                nc.tensor.matmul(vbp[:, :], shift_c[:, ct, :], wvT[:, ct, :],
                                 start=(ct == 0), stop=(ct == CT - 1))
            vb = cst.tile([1, HD], BF16)
            nc.vector.tensor_copy(out=vb, in_=vbp[:, :])

            # ---------- projections ----------
            # q and k duplicated across both partition halves so even/odd
            # k-tiles of the S matmul can use PE row groups (0,0)/(64,0)
            # concurrently (K=64 only half-fills the array)
            q_sb = cst.tile([128, S], BF16)
            k_sb = cst.tile([128, S], BF16)
            for blk in range(S // 512):
                sl = bass.ts(blk, 512)
                qp = pp.tile([HD, 512], F32, tag="pp")
                for ct in range(CT):
                    nc.tensor.matmul(qp[:, :], wq2[:, ct, :], x_sb[:, ct, sl],
                                     start=(ct == 0), stop=(ct == CT - 1))
                nc.vector.tensor_scalar_add(out=q_sb[0:HD, sl], in0=qp[:, :], scalar1=qbc)
                nc.vector.tensor_scalar_add(out=q_sb[HD:128, sl], in0=qp[:, :], scalar1=qbc)
                kp = pp.tile([HD, 512], F32, tag="pp")
                for ct in range(CT):
                    nc.tensor.matmul(kp[:, :], wk2[:, ct, :], c_sb[:, ct, sl],
                                     start=(ct == 0), stop=(ct == CT - 1))
                nc.vector.tensor_scalar_add(out=k_sb[0:HD, sl], in0=kp[:, :], scalar1=kbc)
                nc.vector.tensor_scalar_add(out=k_sb[HD:128, sl], in0=kp[:, :], scalar1=kbc)

            # v^T with an appended ones column (gives softmax denominator for free)
            vT = cst.tile([128, KT, HD + 1], BF16)
            nc.gpsimd.dma_start(
                out=vT[:, :, HD],
                in_=bass.AP(tensor=ones_d, offset=0, ap=[[0, 128], [0, KT]]))
            for st_i in range(KT):
                vp = pp.tile([128, HD], F32, tag="pp")
                for ct in range(CT):
                    nc.tensor.matmul(vp[:, :], c_sb[:, ct, bass.ts(st_i, 128)], wv2[:, ct, :],
                                     start=(ct == 0), stop=False)
                nc.tensor.matmul(vp[:, :], ones_sb[:, 0:128], vb, start=False, stop=True)
                nc.vector.tensor_copy(out=vT[:, st_i, 0:HD], in_=vp[:, :])

            # ---------- attention ----------
            for qsb in range(NQSB):
                o_ps = op.tile([HD + 1, QSB], F32, tag="op")
                for kt in range(KT):
                    s_ps = sp.tile([128, QSB], F32, tag="sp")
                    rg = HD * (kt % 2)
                    for jb in range(JB):
                        nc.tensor.matmul(
                            s_ps[:, bass.ts(jb, 512)],
                            k_sb[rg:rg + HD, bass.ts(kt, 128)],
                            q_sb[rg:rg + HD, bass.ds(qsb * QSB + jb * 512, 512)],
                            start=True, stop=True, tile_position=(rg, 0))
                    p_t = ppool.tile([128, QSB], BF16, tag="ptile")
                    nc.scalar.activation(out=p_t, in_=s_ps[:, :], func=AF.Exp, scale=float(SCALE))
                    for jb in range(JB):
                        nc.tensor.matmul(
                            o_ps[:, bass.ts(jb, 512)],
                            vT[:, kt, :],
                            p_t[:, bass.ts(jb, 512)],
                            start=(kt == 0), stop=(kt == KT - 1))
                # evacuate o (unnormalized) + sums row, freeing the accumulator
                o_raw = work.tile([HD, QSB], BF16, tag="o_raw")
                nc.vector.tensor_copy(out=o_raw, in_=o_ps[0:HD, :])
                ssb = work.tile([1, QSB], F32, tag="ssb")
                nc.vector.tensor_copy(out=ssb, in_=o_ps[HD:HD + 1, :])
                # 1/sums, reshaped to [128, QSB/128] so the DVE divide is wide
                srow = work.tile([128, QSB // 128], F32, tag="srow")
                nc.sync.dma_start(out=srow, in_=ssb.rearrange("p (a b) -> p a b", a=128))
                rsm = work.tile([128, QSB // 128], F32, tag="rsm")
                nc.vector.reciprocal(out=rsm, in_=srow)
                nc.sync.dma_start(
                    out=rscr_d[qsb:qsb + 1, :].rearrange("p (a b) -> p a b", a=128),
                    in_=rsm)
                rb = work.tile([128, QSB], F32, tag="rb")
                nc.sync.dma_start(
                    out=rb,
                    in_=bass.AP(tensor=rscr_d, offset=qsb * QSB, ap=[[0, 128], [1, QSB]]))
                # project (unnormalized), then scale columns by 1/sum at evacuation
                for mt in range(CT):
                    for jb in range(JB):
                        pr = pp.tile([128, 512], F32, tag="pp")
                        nc.tensor.matmul(pr[:, :], woT[:, mt, :], o_raw[:, bass.ts(jb, 512)],
                                         start=True, stop=True)
                        ot = ostage.tile([128, 512], F32, tag="ot")
                        nc.vector.tensor_mul(out=ot, in0=pr[:, :], in1=rb[:, bass.ts(jb, 512)])
                        nc.sync.dma_start(
                            out=out_d[mt * 128:(mt + 1) * 128,
                                      bass.ds(qsb * QSB + jb * 512, 512)],
                            in_=ot)

    nc.compile()
    return nc


def _in_maps(inputs):
    inp = np.asarray(inputs["input"], np.float32)
    cx = np.asarray(inputs["c"], np.float32)
    gn_w = np.asarray(inputs["gn_w"], np.float32).reshape(C, 1)
    gn_b = np.asarray(inputs["gn_b"], np.float32).reshape(C, 1)
    wq = np.asarray(inputs["wq"], np.float32)
    wkv = np.asarray(inputs["wkv"], np.float32)
    wout = np.asarray(inputs["wout_w"], np.float32)

    ch = np.arange(C)
    gagg = np.zeros((CT, 128, NG), np.float32)
    gbc = np.zeros((CT, NG, 128), np.float32)
    for ct in range(CT):
        loc = ch[ct * 128:(ct + 1) * 128]
        for i, c in enumerate(loc):
            g = c // (C // NG)
            gagg[ct, i, g] = 1.0
            gbc[ct, g, i] = 1.0
    ones = np.ones((1, 512), np.float32)

    maps = []
    for core in range(8):
        b, h = core // NH, core % NH
        hs = slice(h * HD, (h + 1) * HD)
        maps.append({
            "x": np.ascontiguousarray(inp[b].reshape(C, S)),
            "cx": np.ascontiguousarray(cx[b].reshape(C, S)),
            "gamma": gn_w, "beta": gn_b,
            "wqT": np.ascontiguousarray(wq[hs, :].T),
            "wkT": np.ascontiguousarray(wkv[h * 2 * HD:h * 2 * HD + HD, :].T),
            "wvT": np.ascontiguousarray(wkv[h * 2 * HD + HD:(h + 1) * 2 * HD, :].T),
            "woT": np.ascontiguousarray(wout[:, hs].T),
            "gagg": gagg, "gbc": gbc, "onesd": ones,
        })
    return maps


def kernel(**inputs):
    global _compiled
    if _compiled is None:
        _compiled = _build()
    nc = _compiled
    maps = _in_maps(inputs)
    res = run_bass_kernel_spmd(nc, maps, list(range(8)))
    wout_b = np.asarray(inputs["wout_b"], np.float32)
    inp = np.asarray(inputs["input"], np.float32)
    out = np.empty((B, C, HH, WW), np.float32)
    for b in range(B):
        acc = res.results[b * NH + 0]["out"].copy()
        for h in range(1, NH):
            acc += res.results[b * NH + h]["out"]
        out[b] = acc.reshape(C, HH, WW) + wout_b[:, None, None] + inp[b]
    return out


# revision 18
# speedup vs baseline: 1.1565x; 1.1565x over previous
"""Cross-attention (GroupNorm + 1x1-conv QKV + softmax attention + out-proj
+ residual) for B=2, C=256, H=W=64, 4 heads, on 8 Trainium2 NeuronCores.

Sharding: core i handles (batch b = i//4, head h = i%4) — data-parallel over
batch x tensor-parallel over heads. Each core computes its head's attention
output projected through its slice of wout columns (a partial sum over the
full [256, 64*64] output); the host sums the 4 head-partials per batch and
adds wout bias + residual input.

Device math per core:
  GN stats via bn_stats per channel + PE group-aggregation matmuls (fp32r);
  the GN affine is folded into the QKV weights (w' = wT * scale_ch, bias
  columns via wT @ shift matmuls) so normalized tensors are never
  materialized. All high-volume matmuls (projections, S^T = k^T q,
  attention*V, out-proj) run bf16 with fp32 PSUM accumulation. S^T is
  computed directly in [k, q] layout; softmax needs no max-subtraction
  (scores are O(1): GN-normalized inputs, 1/sqrt(C) scaling — verified
  |s| < 4) and the denominator comes free from a ones-column appended to
  v^T in the attention-value matmul. The softmax normalization is applied
  after the out-projection (scales commute past the column-wise matmul).
"""

import ml_dtypes
import numpy as np

import concourse.bass as bass
import concourse.tile as tile
from concourse import bacc, mybir
from concourse.bass_utils import run_bass_kernel_spmd

F32 = mybir.dt.float32
F32R = mybir.dt.float32r
BF16 = mybir.dt.bfloat16
AF = mybir.ActivationFunctionType

import os as _os

B, C, HH, WW = 2, 256, 64, 64
NH, HD, NG = 4, 64, 32          # heads, head_dim, groups
S = int(_os.environ.get("K_S", HH * WW))   # 4096 spatial positions
CT = C // 128                    # channel tiles (2)
EPS = 1e-5
SCALE = 1.0 / np.sqrt(np.float32(C))  # attention scale 1/16
KT = S // 128                    # 32 k-tiles
QSB = int(_os.environ.get("K_QSB", 1024))  # q superblock (exp/psum tile width)
NQSB = S // QSB                  # 4
JB = QSB // 512                  # 2 512-blocks per superblock

_compiled = None


def _build():
    nc = bacc.Bacc()

    x_d = nc.dram_tensor("x", [C, S], F32, kind="ExternalInput")
    c_d = nc.dram_tensor("cx", [C, S], F32, kind="ExternalInput")
    gamma_d = nc.dram_tensor("gamma", [C, 1], F32, kind="ExternalInput")
    beta_d = nc.dram_tensor("beta", [C, 1], F32, kind="ExternalInput")
    wqT_d = nc.dram_tensor("wqT", [C, HD], F32, kind="ExternalInput")
    wkT_d = nc.dram_tensor("wkT", [C, HD], F32, kind="ExternalInput")
    wvT_d = nc.dram_tensor("wvT", [C, HD], F32, kind="ExternalInput")
    woT_d = nc.dram_tensor("woT", [HD, C], F32, kind="ExternalInput")
    gagg_d = nc.dram_tensor("gagg", [CT, 128, NG], F32, kind="ExternalInput")
    gbc_d = nc.dram_tensor("gbc", [CT, NG, 128], F32, kind="ExternalInput")
    ones_d = nc.dram_tensor("onesd", [1, 512], F32, kind="ExternalInput")
    onesb_d = nc.dram_tensor("onesb", [128, KT], BF16, kind="ExternalInput")
    onesr_d = nc.dram_tensor("onesr", [1, 512], BF16, kind="ExternalInput")
    out_d = nc.dram_tensor("out", [C, S], F32, kind="ExternalOutput")
    rscr_d = nc.dram_tensor("rscr", [NQSB, QSB], F32)  # recip bounce buffer
    import os as _os
    _DBG = _os.environ.get("KDBG", "0") == "1"
    if _DBG:
        qd_d = nc.dram_tensor("qd", [128, S], F32, kind="ExternalOutput")
        kd_d = nc.dram_tensor("kd", [128, S], F32, kind="ExternalOutput")
        vtd_d = nc.dram_tensor("vtd", [128, KT * (HD + 1)], F32, kind="ExternalOutput")


    with tile.TileContext(nc) as tc:
        with tc.tile_pool(name="cst", bufs=1) as cst, \
             tc.tile_pool(name="work", bufs=2) as work, \
             tc.tile_pool(name="ppool", bufs=5) as ppool, \
             tc.tile_pool(name="ostage", bufs=4) as ostage, \
             tc.tile_pool(name="sp", bufs=2, space="PSUM") as sp, \
             tc.tile_pool(name="op", bufs=1, space="PSUM") as op, \
             tc.tile_pool(name="pp", bufs=2, space="PSUM") as pp, \
             tc.tile_pool(name="dscr", bufs=2, space="DRAM") as dscr:

            # ---------- loads (x/c first: GN stats are the critical path) ----------
            # f32 via fast HWDGE queues (chunked for earlier stats start);
            # bf16 copies for the PE made on ACT/DVE (gpsimd cast-DMA is slow)
            x_f = cst.tile([128, CT, S], F32)
            c_f = cst.tile([128, CT, S], F32)
            for ct in range(CT):
                for ch in range(2):
                    nc.sync.dma_start(out=c_f[:, ct, bass.ts(ch, S // 2)],
                                      in_=c_d[ct * 128:(ct + 1) * 128, bass.ts(ch, S // 2)])
            for ct in range(CT):
                for ch in range(2):
                    nc.sync.dma_start(out=x_f[:, ct, bass.ts(ch, S // 2)],
                                      in_=x_d[ct * 128:(ct + 1) * 128, bass.ts(ch, S // 2)])
            x_sb = cst.tile([128, CT, S], BF16)
            c_sb = cst.tile([128, CT, S], BF16)
            for ct in range(CT):
                nc.scalar.copy(out=c_sb[:, ct, :], in_=c_f[:, ct, :])
            for ct in range(CT):
                nc.scalar.copy(out=x_sb[:, ct, :], in_=x_f[:, ct, :])
            wqT = cst.tile([128, CT, HD], F32R)
            wkT = cst.tile([128, CT, HD], F32R)
            wvT = cst.tile([128, CT, HD], F32R)
            for ct in range(CT):
                nc.gpsimd.dma_start(out=wqT[:, ct, :], in_=wqT_d[ct * 128:(ct + 1) * 128, :])
                nc.gpsimd.dma_start(out=wkT[:, ct, :], in_=wkT_d[ct * 128:(ct + 1) * 128, :])
                nc.gpsimd.dma_start(out=wvT[:, ct, :], in_=wvT_d[ct * 128:(ct + 1) * 128, :])
            woT = cst.tile([HD, CT, 128], BF16)
            for ct in range(CT):
                nc.gpsimd.dma_start(out=woT[:, ct, :], in_=woT_d[:, ct * 128:(ct + 1) * 128])
            gagg = cst.tile([128, CT, NG], F32R)
            gbc = cst.tile([NG, CT, 128], F32R)
            for ct in range(CT):
                nc.gpsimd.dma_start(out=gagg[:, ct, :], in_=gagg_d[ct, :, :])
                nc.gpsimd.dma_start(out=gbc[:, ct, :], in_=gbc_d[ct, :, :])
            ones_sb = cst.tile([1, 512], BF16)
            nc.sync.dma_start(out=ones_sb, in_=onesr_d[:, :])
            gamma_sb = cst.tile([128, CT, 1], F32)
            beta_sb = cst.tile([128, CT, 1], F32)
            for ct in range(CT):
                nc.sync.dma_start(out=gamma_sb[:, ct, :], in_=gamma_d[ct * 128:(ct + 1) * 128, :])
                nc.sync.dma_start(out=beta_sb[:, ct, :], in_=beta_d[ct * 128:(ct + 1) * 128, :])
            eps_t = cst.tile([NG, 1], F32)
            nc.vector.memset(eps_t, EPS)

            # ---------- GroupNorm stats -> per-channel scale/shift ----------
            def gn_affine(src_sb, name):
                # per-channel mean/var over spatial (8 bn_stats subgroups of 512)
                st = cst.tile([128, CT, S // 512, 6], F32, tag=f"st_{name}")
                mv = cst.tile([128, CT, 2], F32, tag=f"mv_{name}")
                mt = cst.tile([128, CT, 2], F32R, tag=f"mt_{name}")
                for ct in range(CT):
                    for sg in range(S // 512):
                        nc.vector.bn_stats(out=st[:, ct, sg, :],
                                           in_=src_sb[:, ct, sg * 512:(sg + 1) * 512])
                    nc.vector.bn_aggr(out=mv[:, ct, :], in_=st[:, ct, :, :])
                    # mt = [m, v + m^2]  (rounded to f32r for the PE aggregation)
                    nc.vector.tensor_mul(out=mt[:, ct, 1:2], in0=mv[:, ct, 0:1], in1=mv[:, ct, 0:1])
                    nc.vector.tensor_add(out=mt[:, ct, 1:2],
                                         in0=mt[:, ct, 1:2].bitcast(F32), in1=mv[:, ct, 1:2])
                    nc.vector.tensor_copy(out=mt[:, ct, 0:1], in_=mv[:, ct, 0:1])
                # group sums over the 8 channels of each group
                gs = pp.tile([NG, 2], F32, tag="pp")
                for ct in range(CT):
                    nc.tensor.matmul(gs[:, :], gagg[:, ct, :], mt[:, ct, :],
                                     start=(ct == 0), stop=(ct == CT - 1))
                grp = cst.tile([NG, 2], F32, tag=f"grp_{name}")
                nc.scalar.mul(out=grp, in_=gs[:, :], mul=1.0 / (C // NG))  # [Mg, E[x^2]g]
                vg = cst.tile([NG, 1], F32, tag=f"vg_{name}")
                nc.vector.tensor_mul(out=vg, in0=grp[:, 0:1], in1=grp[:, 0:1])
                nc.vector.tensor_sub(out=vg, in0=grp[:, 1:2], in1=vg)
                nc.scalar.activation(out=vg, in_=vg, func=AF.Sqrt, bias=eps_t)
                grp2 = cst.tile([NG, 2], F32R, tag=f"grp2_{name}")
                with nc.allow_low_precision(reason="f32r keeps ~fp32 precision"):
                    nc.vector.reciprocal(out=grp2[:, 0:1], in_=vg)  # rstd_g
                nc.vector.tensor_copy(out=grp2[:, 1:2], in_=grp[:, 0:1])  # Mg
                # broadcast group -> channel, then channel affine
                scale = cst.tile([128, CT, 1], F32, tag=f"scale_{name}")
                shift = cst.tile([128, CT, 1], F32R, tag=f"shift_{name}")
                for ct in range(CT):
                    cb = pp.tile([128, 2], F32, tag="pp")
                    nc.tensor.matmul(cb[:, :], gbc[:, ct, :], grp2[:, :],
                                     start=True, stop=True)
                    nc.vector.tensor_mul(out=scale[:, ct, :], in0=gamma_sb[:, ct, :], in1=cb[:, 0:1])
                    tmp = work.tile([128, 1], F32, tag="gn_tmp")
                    nc.vector.tensor_mul(out=tmp, in0=cb[:, 1:2], in1=scale[:, ct, :])
                    nc.vector.tensor_sub(out=shift[:, ct, :], in0=beta_sb[:, ct, :], in1=tmp)
                return scale, shift

            scale_c, shift_c = gn_affine(c_f, "c")
            scale_x, shift_x = gn_affine(x_f, "x")

            # ---------- fold GN affine into weights (bf16 for the PE) ----------
            wq2 = cst.tile([128, CT, HD], BF16)
            wk2 = cst.tile([128, CT, HD], BF16)
            wv2 = cst.tile([128, CT, HD], BF16)
            for ct in range(CT):
                nc.vector.tensor_scalar_mul(out=wq2[:, ct, :], in0=wqT[:, ct, :].bitcast(F32),
                                            scalar1=scale_x[:, ct, :])
                nc.vector.tensor_scalar_mul(out=wk2[:, ct, :], in0=wkT[:, ct, :].bitcast(F32),
                                            scalar1=scale_c[:, ct, :])
                nc.vector.tensor_scalar_mul(out=wv2[:, ct, :], in0=wvT[:, ct, :].bitcast(F32),
                                            scalar1=scale_c[:, ct, :])

            def bias_row(wT, shift, name):
                bp = pp.tile([1, HD], F32, tag="pp")
                for ct in range(CT):
                    nc.tensor.matmul(bp[:, :], shift[:, ct, :], wT[:, ct, :],
                                     start=(ct == 0), stop=(ct == CT - 1))
                brow = cst.tile([1, HD], BF16, tag=f"brow_{name}")
                nc.vector.tensor_copy(out=brow, in_=bp[:, :])
                return brow

            # q/k bias as per-partition columns (added during PSUM evacuation):
            # row matmul -> [1, HD], then a tiny DMA scatter to [HD, 1]
            def bias_col(wT, shift, name):
                bp = pp.tile([1, HD], F32, tag="pp")
                for ct in range(CT):
                    nc.tensor.matmul(bp[:, :], shift[:, ct, :], wT[:, ct, :],
                                     start=(ct == 0), stop=(ct == CT - 1))
                brow = cst.tile([1, HD], F32, tag=f"brow_{name}")
                nc.vector.tensor_copy(out=brow, in_=bp[:, :])
                bcol = cst.tile([HD, 1], F32, tag=f"bcol_{name}")
                nc.sync.dma_start(out=bcol, in_=brow.rearrange("p (a b) -> p a b", a=HD))
                return bcol

            kb = bias_row(wkT, shift_c, "k")
            vb = bias_row(wvT, shift_c, "v")
            qbc = bias_col(wqT, shift_x, "q")

            # ---------- projections ----------
            # q and k duplicated across both partition halves so even/odd
            # k-tiles of the S matmul can use PE row groups (0,0)/(64,0)
            # concurrently (K=64 only half-fills the array). k and v first:
            # attention superblock 0 needs every k-tile and v-tile.
            q_sb = cst.tile([128, S], BF16)
            k_sb = cst.tile([128, S], BF16)
            vT = cst.tile([128, KT, HD + 1], BF16)
            nc.sync.dma_start(out=vT[:, :, HD], in_=onesb_d[:, :])
            for blk in range(S // 512):
                sl = bass.ts(blk, 512)
                kp = pp.tile([HD, 512], F32, tag="pp")
                for ct in range(CT):
                    nc.tensor.matmul(kp[:, :], wk2[:, ct, :], c_sb[:, ct, sl],
                                     start=(ct == 0), stop=False)
                nc.tensor.matmul(kp[:, :], kb, ones_sb, start=False, stop=True)
                nc.scalar.copy(out=k_sb[0:HD, sl], in_=kp[:, :])
                nc.vector.tensor_copy(out=k_sb[HD:128, sl], in_=kp[:, :])
            for st_i in range(KT):
                vp = pp.tile([128, HD], F32, tag="pp")
                for ct in range(CT):
                    nc.tensor.matmul(vp[:, :], c_sb[:, ct, bass.ts(st_i, 128)], wv2[:, ct, :],
                                     start=(ct == 0), stop=False)
                nc.tensor.matmul(vp[:, :], ones_sb[:, 0:128], vb, start=False, stop=True)
                nc.vector.tensor_copy(out=vT[:, st_i, 0:HD], in_=vp[:, :])
            for blk in range(S // 512):
                sl = bass.ts(blk, 512)
                qp = pp.tile([HD, 512], F32, tag="pp")
                for ct in range(CT):
                    nc.tensor.matmul(qp[:, :], wq2[:, ct, :], x_sb[:, ct, sl],
                                     start=(ct == 0), stop=(ct == CT - 1))
                nc.vector.tensor_scalar_add(out=q_sb[0:HD, sl], in0=qp[:, :], scalar1=qbc)
                nc.vector.tensor_scalar_add(out=q_sb[HD:128, sl], in0=qp[:, :], scalar1=qbc)

            if _DBG:
                for ct in range(CT):
                    dmp = work.tile([128, S // 2], F32, tag="dmp")
                    nc.vector.tensor_copy(out=dmp, in_=q_sb[:, bass.ts(ct, S // 2)])
                    nc.sync.dma_start(out=qd_d[:, bass.ts(ct, S // 2)], in_=dmp)
                    dmp2 = work.tile([128, S // 2], F32, tag="dmp")
                    nc.vector.tensor_copy(out=dmp2, in_=k_sb[:, bass.ts(ct, S // 2)])
                    nc.sync.dma_start(out=kd_d[:, bass.ts(ct, S // 2)], in_=dmp2)
                dmp3 = work.tile([128, KT * (HD + 1)], F32, tag="dmp3")
                nc.vector.tensor_copy(out=dmp3, in_=vT.rearrange("p a b -> p (a b)"))
                nc.sync.dma_start(out=vtd_d[:, :], in_=dmp3)

            # ---------- attention ----------
            for qsb in range(NQSB):
                o_ps = op.tile([HD + 1, QSB], F32, tag="op")
                for kt in range(KT):
                    s_ps = sp.tile([128, QSB], F32, tag="sp")
                    rg = HD * (kt % 2)
                    for jb in range(JB):
                        nc.tensor.matmul(
                            s_ps[:, bass.ts(jb, 512)],
                            k_sb[rg:rg + HD, bass.ts(kt, 128)],
                            q_sb[rg:rg + HD, bass.ds(qsb * QSB + jb * 512, 512)],
                            start=True, stop=True, tile_position=(rg, 0))
                    p_t = ppool.tile([128, QSB], BF16, tag="ptile")
                    nc.scalar.activation(out=p_t, in_=s_ps[:, :], func=AF.Exp, scale=float(SCALE))
                    for jb in range(JB):
                        nc.tensor.matmul(
                            o_ps[:, bass.ts(jb, 512)],
                            vT[:, kt, :],
                            p_t[:, bass.ts(jb, 512)],
                            start=(kt == 0), stop=(kt == KT - 1))
                # evacuate o (unnormalized) + sums row, freeing the accumulator
                o_raw = work.tile([HD, QSB], BF16, tag="o_raw")
                nc.vector.tensor_copy(out=o_raw, in_=o_ps[0:HD, :])
                ssb = work.tile([1, QSB], F32, tag="ssb")
                nc.vector.tensor_copy(out=ssb, in_=o_ps[HD:HD + 1, :])
                # 1/sums, reshaped to [128, QSB/128] so the DVE divide is wide,
                # then back to a bf16 row; broadcast across partitions via a
                # K=1 ones-matmul (everything stays in tracked SBUF/PSUM tiles)
                srow = work.tile([128, QSB // 128], F32, tag="srow")
                nc.sync.dma_start(out=srow, in_=ssb.rearrange("p (a b) -> p a b", a=128))
                rsm = work.tile([128, QSB // 128], BF16, tag="rsm")
                with nc.allow_low_precision(reason="softmax denom at bf16"):
                    nc.vector.reciprocal(out=rsm, in_=srow)
                rrow = work.tile([1, QSB], BF16, tag="rrow")
                nc.sync.dma_start(out=rrow.rearrange("p (a b) -> p a b", a=128), in_=rsm)
                # project (unnormalized), then scale columns by 1/sum at evacuation
                for jb in range(JB):
                    rbp = pp.tile([128, 512], F32, tag="pp")
                    nc.tensor.matmul(rbp[:, :], ones_sb[:, 0:128], rrow[:, bass.ts(jb, 512)],
                                     start=True, stop=True)
                    rb = work.tile([128, 512], F32, tag="rb")
                    nc.vector.tensor_copy(out=rb, in_=rbp[:, :])
                    for mt in range(CT):
                        pr = pp.tile([128, 512], F32, tag="pp")
                        nc.tensor.matmul(pr[:, :], woT[:, mt, :], o_raw[:, bass.ts(jb, 512)],
                                         start=True, stop=True)
                        ot = ostage.tile([128, 512], F32, tag="ot")
                        nc.vector.tensor_mul(out=ot, in0=pr[:, :], in1=rb)
                        nc.sync.dma_start(
                            out=out_d[mt * 128:(mt + 1) * 128,
                                      bass.ds(qsb * QSB + jb * 512, 512)],
                            in_=ot)

    nc.compile()
    return nc


def _in_maps(inputs):
    inp = np.asarray(inputs["input"], np.float32)
    cx = np.asarray(inputs["c"], np.float32)
    gn_w = np.asarray(inputs["gn_w"], np.float32).reshape(C, 1)
    gn_b = np.asarray(inputs["gn_b"], np.float32).reshape(C, 1)
    wq = np.asarray(inputs["wq"], np.float32)
    wkv = np.asarray(inputs["wkv"], np.float32)
    wout = np.asarray(inputs["wout_w"], np.float32)

    ch = np.arange(C)
    gagg = np.zeros((CT, 128, NG), np.float32)
    gbc = np.zeros((CT, NG, 128), np.float32)
    for ct in range(CT):
        loc = ch[ct * 128:(ct + 1) * 128]
        for i, c in enumerate(loc):
            g = c // (C // NG)
            gagg[ct, i, g] = 1.0
            gbc[ct, g, i] = 1.0
    ones = np.ones((1, 512), np.float32)

    maps = []
    for core in range(8):
        b, h = core // NH, core % NH
        hs = slice(h * HD, (h + 1) * HD)
        maps.append({
            "x": np.ascontiguousarray(inp[b].reshape(C, S)),
            "cx": np.ascontiguousarray(cx[b].reshape(C, S)),
            "gamma": gn_w, "beta": gn_b,
            "wqT": np.ascontiguousarray(wq[hs, :].T),
            "wkT": np.ascontiguousarray(wkv[h * 2 * HD:h * 2 * HD + HD, :].T),
            "wvT": np.ascontiguousarray(wkv[h * 2 * HD + HD:(h + 1) * 2 * HD, :].T),
            "woT": np.ascontiguousarray(wout[:, hs].T),
            "gagg": gagg, "gbc": gbc, "onesd": ones,
            "onesb": np.ones((128, KT), ml_dtypes.bfloat16),
            "onesr": np.ones((1, 512), ml_dtypes.bfloat16),
        })
    return maps


def kernel(**inputs):
    global _compiled
    if _compiled is None:
        _compiled = _build()
    nc = _compiled
    maps = _in_maps(inputs)
    res = run_bass_kernel_spmd(nc, maps, list(range(8)))
    wout_b = np.asarray(inputs["wout_b"], np.float32)
    inp = np.asarray(inputs["input"], np.float32)
    out = np.empty((B, C, HH, WW), np.float32)
    for b in range(B):
        acc = res.results[b * NH + 0]["out"].copy()
        for h in range(1, NH):
            acc += res.results[b * NH + h]["out"]
        out[b] = acc.reshape(C, HH, WW) + wout_b[:, None, None] + inp[b]
    return out
